# revision 1
# baseline (speedup 1.0000x reference)
"""Causal self-attention Trainium2 Bass kernel.

Problem: B=4, T=2048, D=1024, H=16, head_dim=64.
Sharding: 8 cores = (batch b in 0..3) x (head-group g in 0..1, 8 heads each).
Each core computes a partial projection output for its batch over its 512
model dims; the host sums the two partials per batch (b_proj is fed to the
g==0 core only).

All matmuls run in float32r (TF32-like, full PE rate at N=512).
This environment has a large fixed per-instruction cost, so the kernel
batches DMAs, fuses bias adds into evacuation ops, and keeps instruction
count minimal.
"""

import numpy as np

import concourse.bacc as bacc
import concourse.bass as bass
import concourse.mybir as mybir
import concourse.tile as tile
from concourse.bass_utils import run_bass_kernel_spmd
from concourse.masks import make_identity

F32 = mybir.dt.float32
F32R = mybir.dt.float32r
AF = mybir.ActivationFunctionType

B, T, D, H = 4, 2048, 1024, 16
HD = 64              # head dim
HPC = 8              # heads per core
DC = HPC * HD        # 512 model dims per core
SCALE = 1.0 / np.sqrt(HD)

_NC_CACHE = {}


def build_nc(t=T, reps=1, phases="ABC", no_mask=False, no_norm=False,
             no_exp=False):
    """Build the single-core SPMD program. t = sequence length (for small sims).
    reps>1 repeats the computation (device-time measurement); phases/no_*
    are timing-ablation knobs (wrong numerics when used)."""
    nt = t // 128          # 128-row tiles over time
    nq = t // 512          # 512-col chunks over time
    ng = t // 1024         # 1024-col groups over time
    KC = D // 128          # 8 contraction chunks for qkv
    MQK = DC // 128        # 4 feature tiles for each of q,k

    nc = bacc.Bacc("TRN2", target_bir_lowering=False, debug=False)

    xT_d = nc.dram_tensor("xT", [D, t], F32R, kind="ExternalInput")
    wq_d = nc.dram_tensor("wq", [D, DC], F32R, kind="ExternalInput")
    wk_d = nc.dram_tensor("wk", [D, DC], F32R, kind="ExternalInput")
    wv_d = nc.dram_tensor("wv", [D, DC], F32R, kind="ExternalInput")
    bq_d = nc.dram_tensor("bq", [1, DC], F32, kind="ExternalInput")
    bk_d = nc.dram_tensor("bk", [1, DC], F32, kind="ExternalInput")
    bv_d = nc.dram_tensor("bv", [1, DC], F32, kind="ExternalInput")
    wp_d = nc.dram_tensor("wp", [DC, D], F32R, kind="ExternalInput")
    bp_d = nc.dram_tensor("bp", [1, D], F32, kind="ExternalInput")
    ones_d = nc.dram_tensor("cones", [1, 512], F32R, kind="ExternalInput")
    out_d = nc.dram_tensor("out", [t, D], F32, kind="ExternalOutput")

    with tile.TileContext(nc) as tc:
      for _rep in range(reps):
        with tc.tile_pool(name="persist", bufs=1) as persist, \
             tc.tile_pool(name="vpool", bufs=1) as vpool, \
             tc.tile_pool(name="qkpool", bufs=1) as qkpool:

            # resident qk^T: [:, m, :] = q^T feats tile m, [:, 4+m, :] = k^T
            qkTb = qkpool.tile([128, 2 * MQK, t], F32R)

            # static mask: tmask[p, u] = 1 iff u - p >= 384; slice
            # tmask[:, 384-s:512] masks a diagonal region with offset s
            tmask = persist.tile([128, 512], F32)
            nc.gpsimd.memset(tmask[:], 1.0)
            nc.gpsimd.affine_select(
                out=tmask[:], in_=tmask[:],
                compare_op=mybir.AluOpType.is_ge, fill=0.0,
                base=-384, pattern=[[1, 512]], channel_multiplier=-1)
            # broadcast constants
            ones_bc = persist.tile([128, nt * HPC], F32R)
            nc.gpsimd.dma_start(
                ones_bc[:], ones_d[0:1, 0:nt * HPC].to_broadcast([128, nt * HPC]))
            bv_bc = persist.tile([128, DC], F32)
            nc.gpsimd.dma_start(bv_bc[:], bv_d[0:1, :].to_broadcast([128, DC]))
            bp_bc = persist.tile([128, D], F32)
            nc.gpsimd.dma_start(bp_bc[:], bp_d[0:1, :].to_broadcast([128, D]))
            # partition-major per-feature-tile bias columns [128, MQK]
            bqp = persist.tile([128, MQK], F32)
            nc.sync.dma_start(bqp[:], bq_d.rearrange("o (m p) -> p (o m)", p=128))
            bkp = persist.tile([128, MQK], F32)
            nc.sync.dma_start(bkp[:], bk_d.rearrange("o (m p) -> p (o m)", p=128))

            # v' mega-tile: [128, nt, 8*65]; col h*65+64 holds ones
            vpm = vpool.tile([128, nt, HPC * (HD + 1)], F32R)
            nc.vector.tensor_copy(
                vpm.rearrange("p t (h e) -> p (t h) e", e=HD + 1)[:, :, HD:HD + 1],
                ones_bc[:].unsqueeze(2))

            # ---------------- Phase A: qkv ----------------
            with tc.tile_pool(name="phA_sb", bufs=1) as pa, \
                 tc.tile_pool(name="phA_w", bufs=2) as pw, \
                 tc.tile_pool(name="phA_ps", bufs=2, space="PSUM") as pps:

                # x^T resident: one tile [128, KC, t], single DMA
                xTb = pa.tile([128, KC, t], F32R)
                nc.sync.dma_start(
                    xTb[:], xT_d.rearrange("(k p) t -> p k t", p=128))

                # q^T / k^T -> psum -> (bias-add) resident qkTb
                for sec, (w_d, b_s) in enumerate(
                        ((wq_d, bqp), (wk_d, bkp)) if ("q" in phases or "A" in phases) else ()):
                    ws = pw.tile([128, KC, DC], F32R, name=f"ws{sec}", tag="wsec")
                    nc.sync.dma_start(ws[:], w_d.rearrange("(k p) c -> p k c", p=128))
                    for m in range(MQK):
                        for np2 in range(nq // 2):
                            ps = pps.tile([128, 1024], F32, name="qkps", tag="psqk")
                            for k in range(KC):
                                for half in range(2):
                                    n = 2 * np2 + half
                                    nc.tensor.matmul(
                                        ps[:, half * 512:(half + 1) * 512],
                                        ws[:, k, m * 128:(m + 1) * 128],
                                        xTb[:, k, n * 512:(n + 1) * 512],
                                        start=(k == 0), stop=(k == KC - 1))
                            nc.vector.tensor_scalar_add(
                                qkTb[:, sec * MQK + m,
                                     np2 * 1024:(np2 + 1) * 1024],
                                ps[:], b_s[:, m:m + 1])

                # v natural (+bias) -> strided copy into v' tiles
                if "v" in phases or "A" in phases:
                    wvs = pw.tile([128, KC, DC], F32R, name="wvs", tag="wsec")
                    nc.sync.dma_start(wvs[:], wv_d.rearrange("(k p) c -> p k c", p=128))
                    for tt in range(nt):
                        ps = pps.tile([128, 512], F32, name="vps", tag="psv")
                        for k in range(KC):
                            nc.tensor.matmul(
                                ps[:],
                                xTb[:, k, tt * 128:(tt + 1) * 128],
                                wvs[:, k, :],
                                start=(k == 0), stop=(k == KC - 1))
                        nc.vector.tensor_add(
                            vpm[:, tt].rearrange("p (h e) -> p h e", e=HD + 1)[:, :, 0:HD],
                            ps.rearrange("p (h e) -> p h e", e=HD),
                            bv_bc.rearrange("p (h e) -> p h e", e=HD))

            # ---------------- Phase B: attention ----------------
            if "B" not in phases:
                continue
            with tc.tile_pool(name="yT", bufs=1) as ypool:
                yT = [ypool.tile([128, t], F32R, name=f"yT{f}", tag=f"yT{f}")
                      for f in range(MQK)]

                with tc.tile_pool(name="esb", bufs=2) as pesb, \
                     tc.tile_pool(name="norm", bufs=1) as pnorm, \
                     tc.tile_pool(name="sc_ps", bufs=1, space="PSUM") as pscps, \
                     tc.tile_pool(name="y_ps", bufs=1, space="PSUM") as pyps:

                    for f in range(MQK):
                        for hh in range(2):
                            h = 2 * f + hh
                            qh = qkTb[:, f][hh * HD:(hh + 1) * HD, :]
                            kh = qkTb[:, MQK + f][hh * HD:(hh + 1) * HD, :]
                            y_acc = pyps.tile([HD + 1, t], F32,
                                              name=f"yacc{h}", tag="yacc")
                            for kc in range(nt):
                                nmin = kc // 4
                                dn = kc // 4      # diagonal 512-chunk index
                                s = 128 * (kc % 4)
                                dlo = dn * 512 + s   # first live column
                                sp = pscps.tile([128, t], F32,
                                                name="scps", tag="scps")
                                for n in range(nmin, nq):
                                    w0 = dlo if n == dn else n * 512
                                    nc.tensor.matmul(
                                        sp[:, w0:(n + 1) * 512],
                                        kh[:, kc * 128:(kc + 1) * 128],
                                        qh[:, w0:(n + 1) * 512],
                                        start=True, stop=True)
                                esb = pesb.tile([128, t], F32R,
                                                name="esb", tag="esb")
                                nc.scalar.activation(esb[:, dlo:], sp[:, dlo:],
                                                     AF.Copy if no_exp else AF.Exp,
                                                     scale=float(SCALE))
                                if not no_mask:
                                    nc.vector.tensor_mul(
                                        esb[:, dlo:dlo + 128],
                                        esb[:, dlo:dlo + 128],
                                        tmask[:, 384:512])
                                for n in range(nmin, nq):
                                    w0 = dlo if n == dn else n * 512
                                    nc.tensor.matmul(
                                        y_acc[:, w0:(n + 1) * 512],
                                        vpm[:, kc, h * (HD + 1):(h + 1) * (HD + 1)],
                                        esb[:, w0:(n + 1) * 512],
                                        start=(kc == 0), stop=(kc == 4 * n + 3))
                            # normalize: yT[f][hh*64:, :] = y/denom
                            if no_norm:
                                nc.vector.tensor_copy(
                                    yT[f][hh * HD:(hh + 1) * HD, :],
                                    y_acc[0:HD, :])
                            else:
                                rec = pnorm.tile([1, t], F32, name="rec", tag="rec")
                                nc.vector.reciprocal(rec[:], y_acc[HD:HD + 1, :])
                                rb = pnorm.tile([HD, t], F32, name="rb", tag="rb")
                                nc.gpsimd.partition_broadcast(rb[:], rec[:])
                                nc.vector.tensor_mul(
                                    yT[f][hh * HD:(hh + 1) * HD, :],
                                    y_acc[0:HD, :], rb[:])

                # ---------------- Phase C: projection ----------------
                if "C" not in phases:
                    continue
                with tc.tile_pool(name="phC_sb", bufs=1) as pc, \
                     tc.tile_pool(name="phC_evac", bufs=3) as pcev, \
                     tc.tile_pool(name="phC_ps", bufs=3, space="PSUM") as pcps:
                    wpb = pc.tile([128, MQK, D], F32R)
                    nc.sync.dma_start(
                        wpb[:], wp_d.rearrange("(m p) o -> p m o", p=128))
                    for qtp in range(nt // 2):
                        ev = pcev.tile([128, 2, D], F32, name="prev", tag="prev")
                        for half in range(2):
                            qt = 2 * qtp + half
                            ps = pcps.tile([128, 1024], F32, name="prps", tag="prps")
                            for oc in range(D // 512):
                                for m in range(MQK):
                                    nc.tensor.matmul(
                                        ps[:, oc * 512:(oc + 1) * 512],
                                        yT[m][:, qt * 128:(qt + 1) * 128],
                                        wpb[:, m, oc * 512:(oc + 1) * 512],
                                        start=(m == 0), stop=(m == MQK - 1))
                            nc.vector.tensor_add(ev[:, half, :], ps[:], bp_bc[:])
                        nc.sync.dma_start(
                            out_d[qtp * 256:(qtp + 1) * 256, :]
                            .rearrange("(a p) o -> p a o", p=128),
                            ev[:])

    nc.finalize()
    return nc


def make_in_maps(x, w_attn, b_attn, w_proj, b_proj):
    x = np.ascontiguousarray(np.asarray(x, dtype=np.float32))
    w_attn = np.asarray(w_attn, dtype=np.float32)
    b_attn = np.asarray(b_attn, dtype=np.float32)
    w_proj = np.asarray(w_proj, dtype=np.float32)
    b_proj = np.asarray(b_proj, dtype=np.float32)
    in_maps = []
    for c in range(8):
        b, g = c // 2, c % 2
        sl = slice(DC * g, DC * (g + 1))
        in_maps.append({
            "xT": np.ascontiguousarray(x[b].T),
            "wq": np.ascontiguousarray(w_attn[:, 0 * D:][:, sl]),
            "wk": np.ascontiguousarray(w_attn[:, 1 * D:][:, sl]),
            "wv": np.ascontiguousarray(w_attn[:, 2 * D:][:, sl]),
            "bq": np.ascontiguousarray(b_attn[0 * D:1 * D][sl][None, :]),
            "bk": np.ascontiguousarray(b_attn[1 * D:2 * D][sl][None, :]),
            "bv": np.ascontiguousarray(b_attn[2 * D:3 * D][sl][None, :]),
            "wp": np.ascontiguousarray(w_proj[sl, :]),
            "bp": np.ascontiguousarray(
                (b_proj if g == 0 else np.zeros_like(b_proj))[None, :]),
            "cones": np.ones((1, 512), dtype=np.float32),
        })
    return in_maps


def kernel(x, w_attn, b_attn, w_proj, b_proj, _trace=False, _trace_kwargs=None):
    if "nc" not in _NC_CACHE:
        _NC_CACHE["nc"] = build_nc()
    nc = _NC_CACHE["nc"]
    in_maps = make_in_maps(x, w_attn, b_attn, w_proj, b_proj)
    kw = {}
    if _trace:
        kw["trace"] = True
        if _trace_kwargs:
            kw.update(_trace_kwargs)
    res = run_bass_kernel_spmd(nc, in_maps, core_ids=list(range(8)), **kw)
    outs = [res.results[c]["out"] for c in range(8)]
    out = np.empty((B, T, D), dtype=np.float32)
    for b in range(B):
        np.add(outs[2 * b], outs[2 * b + 1], out=out[b])
    kernel._last_results = res
    return out


if __name__ == "__main__":
    nc = build_nc()
    print("built ok")



# revision 6
# speedup vs baseline: 1532.5687x; 1532.5687x over previous
"""Causal self-attention Trainium2 Bass kernel.

Problem: B=4, T=2048, D=1024, H=16, head_dim=64.
Sharding: 8 cores = (batch b in 0..3) x (head-group g in 0..1, 8 heads each).
Each core computes a partial projection output for its batch over its 512
model dims; the host sums the two partials per batch (b_proj is fed to the
g==0 core only).

Kernel structure (per core):
- Phase A: qkv projections in float32r (full PE rate at N=512); q^T/k^T
  kept resident as [feat, t] tiles, v natural-layout with a ones column
  appended per head (so attn @ v' also yields the softmax denominator).
- Phase B: attention, q-chunk (512 cols) OUTER, key-tile (128) inner.
  Transposed scores [keys, q] go PSUM -> exp (ACT, scale=1/sqrt(hd)) ->
  bf16 esb tiles; causal diagonal blocks are masked by one strided
  tensor_mul per pair; attn @ v' accumulates in a 1-bank PSUM tile.
  Small PSUM tiles (2-banks scores x3 bufs, 1-bank y x2) keep PE and ACT
  streaming concurrently; the qi-outer order lets phase C overlap B.
- Phase C: output projection from resident y^T tiles, bias fused into the
  PSUM evacuation, 2-row-group batched output DMAs.

Timing note: `reps` is implemented as a device-side `tc.For_i` loop (the
body is idempotent, so executing it R times yields the same output).  A
repeat-delta between reps=1 and reps=R therefore measures the marginal
on-device execution time of one body iteration, with the fixed per-call
axon dispatch + NEFF transfer/load overhead cancelled.  Emitting the body
R times unrolled (as earlier revisions did) makes the NEFF size scale
with R, so that delta is dominated by per-instruction NEFF load/transfer
overhead (~100us/emitted instruction) rather than device execution.
"""

import numpy as np

import concourse.bacc as bacc
import concourse.bass as bass
import concourse.mybir as mybir
import concourse.tile as tile
from concourse.bass_utils import run_bass_kernel_spmd

F32 = mybir.dt.float32
F32R = mybir.dt.float32r
BF16 = mybir.dt.bfloat16
AF = mybir.ActivationFunctionType

B, T, D, H = 4, 2048, 1024, 16
HD = 64              # head dim
HPC = 8              # heads per core
DC = HPC * HD        # 512 model dims per core
SCALE = 1.0 / np.sqrt(HD)

_NC_CACHE = {}


def build_nc(t=T, reps=1, phases="ABC", no_mask=False, no_norm=False,
             no_exp=False):
    """Build the single-core SPMD program. t = sequence length (for small
    sims). reps>1 repeats the computation via a device-side For_i loop
    (device-time measurement); phases/no_* are timing-ablation knobs
    (wrong numerics when used)."""
    nt = t // 128          # 128-row tiles over time
    nq = t // 512          # 512-col chunks over time
    KC = D // 128          # 8 contraction chunks for qkv
    MQK = DC // 128        # 4 feature tiles for each of q,k

    nc = bacc.Bacc("TRN2", target_bir_lowering=False, debug=False)

    xT_d = nc.dram_tensor("xT", [D, t], F32R, kind="ExternalInput")
    wq_d = nc.dram_tensor("wq", [D, DC], F32R, kind="ExternalInput")
    wk_d = nc.dram_tensor("wk", [D, DC], F32R, kind="ExternalInput")
    wv_d = nc.dram_tensor("wv", [D, DC], F32R, kind="ExternalInput")
    bq_d = nc.dram_tensor("bq", [1, DC], F32, kind="ExternalInput")
    bk_d = nc.dram_tensor("bk", [1, DC], F32, kind="ExternalInput")
    bv_d = nc.dram_tensor("bv", [1, DC], F32, kind="ExternalInput")
    wp_d = nc.dram_tensor("wp", [DC, D], F32R, kind="ExternalInput")
    bp_d = nc.dram_tensor("bp", [1, D], F32, kind="ExternalInput")
    ones_d = nc.dram_tensor("cones", [1, 512], F32R, kind="ExternalInput")
    out_d = nc.dram_tensor("out", [t, D], F32, kind="ExternalOutput")

    with tile.TileContext(nc) as tc:
      with tc.For_i(0, reps, 1) as _i:
        with tc.tile_pool(name="persist", bufs=1) as persist, \
             tc.tile_pool(name="vpool", bufs=1) as vpool, \
             tc.tile_pool(name="qkpool", bufs=1) as qkpool:

            # resident qk^T: [:, m, :] = q^T feats tile m, [:, 4+m, :] = k^T
            qkTb = qkpool.tile([128, 2 * MQK, t], F32R)

            # tmask[p, c] = 1 iff c >= p (keep lower triangle in [k, q])
            tmask = persist.tile([128, 128], BF16)
            nc.gpsimd.memset(tmask[:], 1.0)
            nc.gpsimd.affine_select(
                out=tmask[:], in_=tmask[:],
                compare_op=mybir.AluOpType.is_ge, fill=0.0,
                base=0, pattern=[[1, 128]], channel_multiplier=-1)
            # tmask2: 2 replicas so one strided mul masks a diagonal pair
            tmask2 = persist.tile([128, 2, 128], BF16)
            for r in range(2):
                nc.vector.tensor_copy(tmask2[:, r, :], tmask[:])

            ones_bc = persist.tile([128, nt * HPC], BF16)
            nc.gpsimd.dma_start(
                ones_bc[:], ones_d[0:1, 0:nt * HPC].to_broadcast([128, nt * HPC]))
            bv_bc = persist.tile([128, DC], F32)
            nc.gpsimd.dma_start(bv_bc[:], bv_d[0:1, :].to_broadcast([128, DC]))
            bp_bc = persist.tile([128, D], F32)
            nc.gpsimd.dma_start(bp_bc[:], bp_d[0:1, :].to_broadcast([128, D]))
            # partition-major per-feature-tile bias columns [128, MQK]
            bqp = persist.tile([128, MQK], F32)
            nc.sync.dma_start(bqp[:], bq_d.rearrange("o (m p) -> p (o m)", p=128))
            bkp = persist.tile([128, MQK], F32)
            nc.sync.dma_start(bkp[:], bk_d.rearrange("o (m p) -> p (o m)", p=128))

            # v' mega-tile (bf16): [128, nt, 8*65]; col h*65+64 holds ones
            vpm = vpool.tile([128, nt, HPC * (HD + 1)], BF16)
            nc.vector.tensor_copy(
                vpm.rearrange("p t (h e) -> p (t h) e", e=HD + 1)[:, :, HD:HD + 1],
                ones_bc[:].unsqueeze(2))

            # ---------------- Phase A: qkv ----------------
            with tc.tile_pool(name="phA_sb", bufs=1) as pa, \
                 tc.tile_pool(name="phA_w", bufs=2) as pw, \
                 tc.tile_pool(name="phA_ps", bufs=2, space="PSUM") as pps:

                # x^T resident: per-k-chunk DMAs so matmuls start early
                xTb = pa.tile([128, KC, t], F32R)
                for k in range(KC):
                    nc.sync.dma_start(
                        xTb[:, k, :], xT_d[k * 128:(k + 1) * 128, :])

                # q^T / k^T -> psum -> (bias-add) resident qkTb
                for sec, (w_d, b_s) in enumerate(
                        ((wq_d, bqp), (wk_d, bkp))
                        if ("q" in phases or "A" in phases) else ()):
                    ws = pw.tile([128, KC, DC], F32R, name=f"ws{sec}", tag="wsec")
                    nc.sync.dma_start(ws[:], w_d.rearrange("(k p) c -> p k c", p=128))
                    for m in range(MQK):
                        for np2 in range(nq // 2):
                            ps = pps.tile([128, 1024], F32, name="qkps", tag="psqk")
                            for k in range(KC):
                                for half in range(2):
                                    n = 2 * np2 + half
                                    nc.tensor.matmul(
                                        ps[:, half * 512:(half + 1) * 512],
                                        ws[:, k, m * 128:(m + 1) * 128],
                                        xTb[:, k, n * 512:(n + 1) * 512],
                                        start=(k == 0), stop=(k == KC - 1))
                            nc.vector.tensor_scalar_add(
                                qkTb[:, sec * MQK + m,
                                     np2 * 1024:(np2 + 1) * 1024],
                                ps[:], b_s[:, m:m + 1])

                # v natural (+bias) -> strided copy into v' tiles (bf16)
                if "v" in phases or "A" in phases:
                    wvs = pw.tile([128, KC, DC], F32R, name="wvs", tag="wsec")
                    nc.sync.dma_start(wvs[:], wv_d.rearrange("(k p) c -> p k c", p=128))
                    for tt in range(nt):
                        ps = pps.tile([128, 512], F32, name="vps", tag="psv")
                        for k in range(KC):
                            nc.tensor.matmul(
                                ps[:],
                                xTb[:, k, tt * 128:(tt + 1) * 128],
                                wvs[:, k, :],
                                start=(k == 0), stop=(k == KC - 1))
                        nc.vector.tensor_add(
                            vpm[:, tt].rearrange("p (h e) -> p h e", e=HD + 1)[:, :, 0:HD],
                            ps.rearrange("p (h e) -> p h e", e=HD),
                            bv_bc.rearrange("p (h e) -> p h e", e=HD))

            # -------- Phase B: attention (qi outer, heads inner) --------
            if "B" in phases:
              with tc.tile_pool(name="yT", bufs=1) as ypool:
                yT = [ypool.tile([128, t], F32R, name=f"yT{f}", tag=f"yT{f}")
                      for f in range(MQK)]

                with tc.tile_pool(name="esb", bufs=2) as pesb, \
                     tc.tile_pool(name="norm", bufs=2) as pnorm, \
                     tc.tile_pool(name="sc_ps", bufs=3, space="PSUM") as pscps, \
                     tc.tile_pool(name="y_ps", bufs=2, space="PSUM") as pyps:

                    for qi in range(nq):
                        nkc = 4 * qi + 4
                        for f in range(MQK):
                            for hh in range(2):
                                h = 2 * f + hh
                                qh = qkTb[:, f][hh * HD:(hh + 1) * HD, :]
                                kh = qkTb[:, MQK + f][hh * HD:(hh + 1) * HD, :]
                                # exp'd transposed scores for this q-chunk:
                                # esb[:, kc, :] = exp(k_tile_kc^T q_chunk)
                                esb = pesb.tile([128, nt, 512], BF16,
                                                name="esb", tag="esb")
                                y_acc = pyps.tile([HD + 1, 512], F32,
                                                  name=f"yacc{h}_{qi}",
                                                  tag="yacc")
                                qs = qh[:, qi * 512:(qi + 1) * 512]
                                for kc2 in range(nkc // 2):
                                    sp = pscps.tile([128, 2, 512], F32,
                                                    name="scps", tag="scps")
                                    for half in range(2):
                                        kc = 2 * kc2 + half
                                        nc.tensor.matmul(
                                            sp[:, half, :],
                                            kh[:, kc * 128:(kc + 1) * 128],
                                            qs,
                                            start=True, stop=True)
                                    nc.scalar.activation(
                                        esb[:, 2 * kc2:2 * kc2 + 2, :],
                                        sp[:],
                                        AF.Copy if no_exp else AF.Exp,
                                        scale=float(SCALE))
                                    if kc2 >= 2 * qi and not no_mask:
                                        # diagonal pair: mask two triangle
                                        # blocks (cols (4qi+r)*512 + 128r)
                                        # in one strided op
                                        r0 = 2 * (kc2 - 2 * qi)
                                        diag = bass.AP(
                                            tensor=esb.tensor,
                                            offset=esb.offset
                                            + (4 * qi + r0) * 512 + 128 * r0,
                                            ap=[list(esb[:].ap[0]),
                                                [640, 2], [1, 128]])
                                        nc.vector.tensor_mul(
                                            diag, diag, tmask2[:])
                                for kc in range(nkc):
                                    # diagonal tiles: cols [0, 128r) are
                                    # fully masked (q < k) - skip them
                                    r = kc - 4 * qi
                                    w0 = 128 * r if r > 0 else 0
                                    nc.tensor.matmul(
                                        y_acc[:, w0:],
                                        vpm[:, kc, h * (HD + 1):(h + 1) * (HD + 1)],
                                        esb[:, kc, w0:],
                                        start=(kc == 0), stop=(kc == nkc - 1))
                                # normalize: yT slice = y/denom
                                if no_norm:
                                    nc.vector.tensor_copy(
                                        yT[f][hh * HD:(hh + 1) * HD,
                                              qi * 512:(qi + 1) * 512],
                                        y_acc[0:HD, :])
                                else:
                                    rec = pnorm.tile([1, 512], F32,
                                                     name="rec", tag="rec")
                                    nc.vector.reciprocal(
                                        rec[:], y_acc[HD:HD + 1, :])
                                    rb = pnorm.tile([HD, 512], F32,
                                                    name="rb", tag="rb")
                                    nc.gpsimd.partition_broadcast(rb[:], rec[:])
                                    nc.vector.tensor_mul(
                                        yT[f][hh * HD:(hh + 1) * HD,
                                              qi * 512:(qi + 1) * 512],
                                        y_acc[0:HD, :], rb[:])

                # ---------------- Phase C: projection ----------------
                if "C" in phases:
                  with tc.tile_pool(name="phC_sb", bufs=1) as pc, \
                       tc.tile_pool(name="phC_evac", bufs=3) as pcev, \
                       tc.tile_pool(name="phC_ps", bufs=3, space="PSUM") as pcps:
                    wpb = pc.tile([128, MQK, D], F32R)
                    nc.sync.dma_start(
                        wpb[:], wp_d.rearrange("(m p) o -> p m o", p=128))
                    for qtp in range(nt // 2):
                        ev = pcev.tile([128, 2, D], F32, name="prev", tag="prev")
                        for half in range(2):
                            qt = 2 * qtp + half
                            ps = pcps.tile([128, 1024], F32, name="prps", tag="prps")
                            for oc in range(D // 512):
                                for m in range(MQK):
                                    nc.tensor.matmul(
                                        ps[:, oc * 512:(oc + 1) * 512],
                                        yT[m][:, qt * 128:(qt + 1) * 128],
                                        wpb[:, m, oc * 512:(oc + 1) * 512],
                                        start=(m == 0), stop=(m == MQK - 1))
                            nc.vector.tensor_add(ev[:, half, :], ps[:], bp_bc[:])
                        nc.sync.dma_start(
                            out_d[qtp * 256:(qtp + 1) * 256, :]
                            .rearrange("(a p) o -> p a o", p=128),
                            ev[:])

    nc.finalize()
    return nc


def make_in_maps(x, w_attn, b_attn, w_proj, b_proj):
    x = np.ascontiguousarray(np.asarray(x, dtype=np.float32))
    w_attn = np.asarray(w_attn, dtype=np.float32)
    b_attn = np.asarray(b_attn, dtype=np.float32)
    w_proj = np.asarray(w_proj, dtype=np.float32)
    b_proj = np.asarray(b_proj, dtype=np.float32)
    in_maps = []
    for c in range(8):
        b, g = c // 2, c % 2
        sl = slice(DC * g, DC * (g + 1))
        in_maps.append({
            "xT": np.ascontiguousarray(x[b].T),
            "wq": np.ascontiguousarray(w_attn[:, 0 * D:][:, sl]),
            "wk": np.ascontiguousarray(w_attn[:, 1 * D:][:, sl]),
            "wv": np.ascontiguousarray(w_attn[:, 2 * D:][:, sl]),
            "bq": np.ascontiguousarray(b_attn[0 * D:1 * D][sl][None, :]),
            "bk": np.ascontiguousarray(b_attn[1 * D:2 * D][sl][None, :]),
            "bv": np.ascontiguousarray(b_attn[2 * D:3 * D][sl][None, :]),
            "wp": np.ascontiguousarray(w_proj[sl, :]),
            "bp": np.ascontiguousarray(
                (b_proj if g == 0 else np.zeros_like(b_proj))[None, :]),
            "cones": np.ones((1, 512), dtype=np.float32),
        })
    return in_maps


def kernel(x, w_attn, b_attn, w_proj, b_proj, _trace=False, _trace_kwargs=None):
    if "nc" not in _NC_CACHE:
        _NC_CACHE["nc"] = build_nc()
    nc = _NC_CACHE["nc"]
    in_maps = make_in_maps(x, w_attn, b_attn, w_proj, b_proj)
    kw = {}
    if _trace:
        kw["trace"] = True
        if _trace_kwargs:
            kw.update(_trace_kwargs)
    res = run_bass_kernel_spmd(nc, in_maps, core_ids=list(range(8)), **kw)
    outs = [res.results[c]["out"] for c in range(8)]
    out = np.empty((B, T, D), dtype=np.float32)
    for b in range(B):
        np.add(outs[2 * b], outs[2 * b + 1], out=out[b])
    kernel._last_results = res
    return out


if __name__ == "__main__":
    nc = build_nc()
    print("built ok")


# revision 7
# speedup vs baseline: 1723.5995x; 1.1246x over previous
"""Causal self-attention Trainium2 Bass kernel.

Problem: B=4, T=2048, D=1024, H=16, head_dim=64.
Sharding: 8 cores = (batch b in 0..3) x (head-group g in 0..1, 8 heads each).
Each core computes a partial projection output for its batch over its 512
model dims; the host sums the two partials per batch (b_proj is fed to the
g==0 core only).

Kernel structure (per core):
- Phase A: qkv projections in float32r (full PE rate at N=512); q^T/k^T
  kept resident as [feat, t] tiles, v natural-layout with a ones column
  appended per head (so attn @ v' also yields the softmax denominator).
- Phase B: attention, q-chunk (512 cols) OUTER, key-tile (128) inner.
  Transposed scores [keys, q] go PSUM -> exp (ACT, scale=1/sqrt(hd)) ->
  bf16 esb tiles; causal diagonal blocks are masked by one strided
  tensor_mul per pair; attn @ v' accumulates in a 1-bank PSUM tile.
  Small PSUM tiles (2-banks scores x3 bufs, 1-bank y x2) keep PE and ACT
  streaming concurrently; the qi-outer order lets phase C overlap B.
- Phase C: output projection from resident y^T tiles, bias fused into the
  PSUM evacuation, 2-row-group batched output DMAs.

Timing note: `reps` is implemented as a device-side `tc.For_i` loop (the
body is idempotent, so executing it R times yields the same output).  A
repeat-delta between reps=1 and reps=R therefore measures the marginal
on-device execution time of one body iteration, with the fixed per-call
axon dispatch + NEFF transfer/load overhead cancelled.  Emitting the body
R times unrolled (as earlier revisions did) makes the NEFF size scale
with R, so that delta is dominated by per-instruction NEFF load/transfer
overhead (~100us/emitted instruction) rather than device execution.
"""

import numpy as np

import concourse.bacc as bacc
import concourse.bass as bass
import concourse.mybir as mybir
import concourse.tile as tile
from concourse.bass_utils import run_bass_kernel_spmd

F32 = mybir.dt.float32
F32R = mybir.dt.float32r
BF16 = mybir.dt.bfloat16
AF = mybir.ActivationFunctionType

B, T, D, H = 4, 2048, 1024, 16
HD = 64              # head dim
HPC = 8              # heads per core
DC = HPC * HD        # 512 model dims per core
SCALE = 1.0 / np.sqrt(HD)

_NC_CACHE = {}


def build_nc(t=T, reps=1, phases="ABC", no_mask=False, no_norm=False,
             no_exp=False):
    """Build the single-core SPMD program. t = sequence length (for small
    sims). reps>1 repeats the computation via a device-side For_i loop
    (device-time measurement); phases/no_* are timing-ablation knobs
    (wrong numerics when used)."""
    nt = t // 128          # 128-row tiles over time
    nq = t // 512          # 512-col chunks over time
    KC = D // 128          # 8 contraction chunks for qkv
    MQK = DC // 128        # 4 feature tiles for each of q,k

    nc = bacc.Bacc("TRN2", target_bir_lowering=False, debug=False)

    xT_d = nc.dram_tensor("xT", [D, t], F32R, kind="ExternalInput")
    wq_d = nc.dram_tensor("wq", [D, DC], F32R, kind="ExternalInput")
    wk_d = nc.dram_tensor("wk", [D, DC], F32R, kind="ExternalInput")
    wv_d = nc.dram_tensor("wv", [D, DC], F32R, kind="ExternalInput")
    bq_d = nc.dram_tensor("bq", [1, DC], F32, kind="ExternalInput")
    bk_d = nc.dram_tensor("bk", [1, DC], F32, kind="ExternalInput")
    bv_d = nc.dram_tensor("bv", [1, DC], F32, kind="ExternalInput")
    wp_d = nc.dram_tensor("wp", [DC, D], F32R, kind="ExternalInput")
    bp_d = nc.dram_tensor("bp", [1, D], F32, kind="ExternalInput")
    ones_d = nc.dram_tensor("cones", [1, 512], F32R, kind="ExternalInput")
    out_d = nc.dram_tensor("out", [t, D], F32, kind="ExternalOutput")

    with tile.TileContext(nc) as tc:
      with tc.For_i(0, reps, 1) as _i:
        with tc.tile_pool(name="persist", bufs=1) as persist, \
             tc.tile_pool(name="vpool", bufs=1) as vpool, \
             tc.tile_pool(name="qkpool", bufs=1) as qkpool:

            # resident qk^T: [:, m, :] = q^T feats tile m, [:, 4+m, :] = k^T
            qkTb = qkpool.tile([128, 2 * MQK, t], F32R)

            # tmask[p, c] = 1 iff c >= p (keep lower triangle in [k, q])
            tmask = persist.tile([128, 128], BF16)
            nc.gpsimd.memset(tmask[:], 1.0)
            nc.gpsimd.affine_select(
                out=tmask[:], in_=tmask[:],
                compare_op=mybir.AluOpType.is_ge, fill=0.0,
                base=0, pattern=[[1, 128]], channel_multiplier=-1)
            # tmask2: 2 replicas so one strided mul masks a diagonal pair
            tmask2 = persist.tile([128, 2, 128], BF16)
            for r in range(2):
                nc.vector.tensor_copy(tmask2[:, r, :], tmask[:])

            ones_bc = persist.tile([128, nt * HPC], BF16)
            nc.gpsimd.dma_start(
                ones_bc[:], ones_d[0:1, 0:nt * HPC].to_broadcast([128, nt * HPC]))
            bv_bc = persist.tile([128, DC], F32)
            nc.gpsimd.dma_start(bv_bc[:], bv_d[0:1, :].to_broadcast([128, DC]))
            bp_bc = persist.tile([128, D], F32)
            nc.gpsimd.dma_start(bp_bc[:], bp_d[0:1, :].to_broadcast([128, D]))
            # partition-major per-feature-tile bias columns [128, MQK]
            bqp = persist.tile([128, MQK], F32)
            nc.sync.dma_start(bqp[:], bq_d.rearrange("o (m p) -> p (o m)", p=128))
            bkp = persist.tile([128, MQK], F32)
            nc.sync.dma_start(bkp[:], bk_d.rearrange("o (m p) -> p (o m)", p=128))

            # v' mega-tile (bf16): [128, nt, 8*65]; col h*65+64 holds ones
            vpm = vpool.tile([128, nt, HPC * (HD + 1)], BF16)
            nc.vector.tensor_copy(
                vpm.rearrange("p t (h e) -> p (t h) e", e=HD + 1)[:, :, HD:HD + 1],
                ones_bc[:].unsqueeze(2))

            # ---------------- Phase A: qkv ----------------
            with tc.tile_pool(name="phA_sb", bufs=1) as pa, \
                 tc.tile_pool(name="phA_w", bufs=2) as pw, \
                 tc.tile_pool(name="phA_ps", bufs=2, space="PSUM") as pps:

                # x^T resident: per-k-chunk DMAs so matmuls start early
                xTb = pa.tile([128, KC, t], F32R)
                for k in range(KC):
                    nc.sync.dma_start(
                        xTb[:, k, :], xT_d[k * 128:(k + 1) * 128, :])

                # q^T / k^T -> psum -> (bias-add) resident qkTb
                for sec, (w_d, b_s) in enumerate(
                        ((wq_d, bqp), (wk_d, bkp))
                        if ("q" in phases or "A" in phases) else ()):
                    ws = pw.tile([128, KC, DC], F32R, name=f"ws{sec}", tag="wsec")
                    nc.sync.dma_start(ws[:], w_d.rearrange("(k p) c -> p k c", p=128))
                    for m in range(MQK):
                        for np2 in range(nq // 2):
                            ps = pps.tile([128, 1024], F32, name="qkps", tag="psqk")
                            for k in range(KC):
                                for half in range(2):
                                    n = 2 * np2 + half
                                    nc.tensor.matmul(
                                        ps[:, half * 512:(half + 1) * 512],
                                        ws[:, k, m * 128:(m + 1) * 128],
                                        xTb[:, k, n * 512:(n + 1) * 512],
                                        start=(k == 0), stop=(k == KC - 1))
                            nc.vector.tensor_scalar_add(
                                qkTb[:, sec * MQK + m,
                                     np2 * 1024:(np2 + 1) * 1024],
                                ps[:], b_s[:, m:m + 1])

                # v natural (+bias) -> strided copy into v' tiles (bf16)
                if "v" in phases or "A" in phases:
                    wvs = pw.tile([128, KC, DC], F32R, name="wvs", tag="wsec")
                    nc.sync.dma_start(wvs[:], wv_d.rearrange("(k p) c -> p k c", p=128))
                    for tt in range(nt):
                        ps = pps.tile([128, 512], F32, name="vps", tag="psv")
                        for k in range(KC):
                            nc.tensor.matmul(
                                ps[:],
                                xTb[:, k, tt * 128:(tt + 1) * 128],
                                wvs[:, k, :],
                                start=(k == 0), stop=(k == KC - 1))
                        nc.vector.tensor_add(
                            vpm[:, tt].rearrange("p (h e) -> p h e", e=HD + 1)[:, :, 0:HD],
                            ps.rearrange("p (h e) -> p h e", e=HD),
                            bv_bc.rearrange("p (h e) -> p h e", e=HD))

            # -------- Phase B: attention (qi outer, heads inner) --------
            if "B" in phases:
              with tc.tile_pool(name="yT", bufs=1) as ypool:
                yT = [ypool.tile([128, t], F32R, name=f"yT{f}", tag=f"yT{f}")
                      for f in range(MQK)]

                with tc.tile_pool(name="esb", bufs=2) as pesb, \
                     tc.tile_pool(name="norm", bufs=2) as pnorm, \
                     tc.tile_pool(name="sc_ps", bufs=3, space="PSUM") as pscps, \
                     tc.tile_pool(name="y_ps", bufs=2, space="PSUM") as pyps:

                    for qi in range(nq):
                        nkc = 4 * qi + 4
                        for f in range(MQK):
                            for hh in range(2):
                                h = 2 * f + hh
                                qh = qkTb[:, f][hh * HD:(hh + 1) * HD, :]
                                kh = qkTb[:, MQK + f][hh * HD:(hh + 1) * HD, :]
                                # exp'd transposed scores for this q-chunk:
                                # esb[:, kc, :] = exp(k_tile_kc^T q_chunk)
                                esb = pesb.tile([128, nt, 512], BF16,
                                                name="esb", tag="esb")
                                y_acc = pyps.tile([HD + 1, 512], F32,
                                                  name=f"yacc{h}_{qi}",
                                                  tag="yacc")
                                qs = qh[:, qi * 512:(qi + 1) * 512]
                                for kc2 in range(nkc // 2):
                                    sp = pscps.tile([128, 2, 512], F32,
                                                    name="scps", tag="scps")
                                    for half in range(2):
                                        kc = 2 * kc2 + half
                                        # diagonal tiles: cols [0, 128r)
                                        # are fully masked; don't compute
                                        # them (attn-v skips them too, and
                                        # exp of the stale PSUM there is
                                        # finite and never consumed)
                                        r = kc - 4 * qi
                                        w0 = 128 * r if r > 0 else 0
                                        nc.tensor.matmul(
                                            sp[:, half, w0:],
                                            kh[:, kc * 128:(kc + 1) * 128],
                                            qs[:, w0:],
                                            start=True, stop=True)
                                    nc.scalar.activation(
                                        esb[:, 2 * kc2:2 * kc2 + 2, :],
                                        sp[:],
                                        AF.Copy if no_exp else AF.Exp,
                                        scale=float(SCALE))
                                    if kc2 >= 2 * qi and not no_mask:
                                        # diagonal pair: mask two triangle
                                        # blocks (cols (4qi+r)*512 + 128r)
                                        # in one strided op
                                        r0 = 2 * (kc2 - 2 * qi)
                                        diag = bass.AP(
                                            tensor=esb.tensor,
                                            offset=esb.offset
                                            + (4 * qi + r0) * 512 + 128 * r0,
                                            ap=[list(esb[:].ap[0]),
                                                [640, 2], [1, 128]])
                                        nc.vector.tensor_mul(
                                            diag, diag, tmask2[:])
                                for kc in range(nkc):
                                    # diagonal tiles: cols [0, 128r) are
                                    # fully masked (q < k) - skip them
                                    r = kc - 4 * qi
                                    w0 = 128 * r if r > 0 else 0
                                    nc.tensor.matmul(
                                        y_acc[:, w0:],
                                        vpm[:, kc, h * (HD + 1):(h + 1) * (HD + 1)],
                                        esb[:, kc, w0:],
                                        start=(kc == 0), stop=(kc == nkc - 1))
                                # normalize: yT slice = y/denom
                                if no_norm:
                                    nc.vector.tensor_copy(
                                        yT[f][hh * HD:(hh + 1) * HD,
                                              qi * 512:(qi + 1) * 512],
                                        y_acc[0:HD, :])
                                else:
                                    rec = pnorm.tile([1, 512], F32,
                                                     name="rec", tag="rec")
                                    nc.vector.reciprocal(
                                        rec[:], y_acc[HD:HD + 1, :])
                                    rb = pnorm.tile([HD, 512], F32,
                                                    name="rb", tag="rb")
                                    nc.gpsimd.partition_broadcast(rb[:], rec[:])
                                    nc.vector.tensor_mul(
                                        yT[f][hh * HD:(hh + 1) * HD,
                                              qi * 512:(qi + 1) * 512],
                                        y_acc[0:HD, :], rb[:])

                # ---------------- Phase C: projection ----------------
                if "C" in phases:
                  with tc.tile_pool(name="phC_sb", bufs=1) as pc, \
                       tc.tile_pool(name="phC_evac", bufs=3) as pcev, \
                       tc.tile_pool(name="phC_ps", bufs=3, space="PSUM") as pcps:
                    wpb = pc.tile([128, MQK, D], F32R)
                    nc.sync.dma_start(
                        wpb[:], wp_d.rearrange("(m p) o -> p m o", p=128))
                    for qtp in range(nt // 2):
                        ev = pcev.tile([128, 2, D], F32, name="prev", tag="prev")
                        for half in range(2):
                            qt = 2 * qtp + half
                            ps = pcps.tile([128, 1024], F32, name="prps", tag="prps")
                            for oc in range(D // 512):
                                for m in range(MQK):
                                    nc.tensor.matmul(
                                        ps[:, oc * 512:(oc + 1) * 512],
                                        yT[m][:, qt * 128:(qt + 1) * 128],
                                        wpb[:, m, oc * 512:(oc + 1) * 512],
                                        start=(m == 0), stop=(m == MQK - 1))
                            nc.vector.tensor_add(ev[:, half, :], ps[:], bp_bc[:])
                        nc.sync.dma_start(
                            out_d[qtp * 256:(qtp + 1) * 256, :]
                            .rearrange("(a p) o -> p a o", p=128),
                            ev[:])

    nc.finalize()
    return nc


def make_in_maps(x, w_attn, b_attn, w_proj, b_proj):
    x = np.ascontiguousarray(np.asarray(x, dtype=np.float32))
    w_attn = np.asarray(w_attn, dtype=np.float32)
    b_attn = np.asarray(b_attn, dtype=np.float32)
    w_proj = np.asarray(w_proj, dtype=np.float32)
    b_proj = np.asarray(b_proj, dtype=np.float32)
    in_maps = []
    for c in range(8):
        b, g = c // 2, c % 2
        sl = slice(DC * g, DC * (g + 1))
        in_maps.append({
            "xT": np.ascontiguousarray(x[b].T),
            "wq": np.ascontiguousarray(w_attn[:, 0 * D:][:, sl]),
            "wk": np.ascontiguousarray(w_attn[:, 1 * D:][:, sl]),
            "wv": np.ascontiguousarray(w_attn[:, 2 * D:][:, sl]),
            "bq": np.ascontiguousarray(b_attn[0 * D:1 * D][sl][None, :]),
            "bk": np.ascontiguousarray(b_attn[1 * D:2 * D][sl][None, :]),
            "bv": np.ascontiguousarray(b_attn[2 * D:3 * D][sl][None, :]),
            "wp": np.ascontiguousarray(w_proj[sl, :]),
            "bp": np.ascontiguousarray(
                (b_proj if g == 0 else np.zeros_like(b_proj))[None, :]),
            "cones": np.ones((1, 512), dtype=np.float32),
        })
    return in_maps


def kernel(x, w_attn, b_attn, w_proj, b_proj, _trace=False, _trace_kwargs=None):
    if "nc" not in _NC_CACHE:
        _NC_CACHE["nc"] = build_nc()
    nc = _NC_CACHE["nc"]
    in_maps = make_in_maps(x, w_attn, b_attn, w_proj, b_proj)
    kw = {}
    if _trace:
        kw["trace"] = True
        if _trace_kwargs:
            kw.update(_trace_kwargs)
    res = run_bass_kernel_spmd(nc, in_maps, core_ids=list(range(8)), **kw)
    outs = [res.results[c]["out"] for c in range(8)]
    out = np.empty((B, T, D), dtype=np.float32)
    for b in range(B):
        np.add(outs[2 * b], outs[2 * b + 1], out=out[b])
    kernel._last_results = res
    return out


if __name__ == "__main__":
    nc = build_nc()
    print("built ok")
